# revision 25
# baseline (speedup 1.0000x reference)
"""Bilateral filter (5x5, sigma_spatial=1.0, sigma_range=0.1) on 8 trn2 cores.

Data parallel: the (4,3,512,512) input is reflect-padded on the host and cut
into 1024 blocks of 32x32 pixels (stored with a 2-px halo -> 36x36 patches,
x3 channels); each core owns 128 blocks = one SBUF partition per block.

Math per core (channel-major, symmetric tap pairs): for each of the 12
"positive" offsets delta the range weight is computed once on the padded
grid and reused for the mirrored tap (w_{-d}(n) = w_d(n-d), and the spatial
kernel is symmetric):

    d  = x[n+delta] - x[n]                    (DVE/GPSIMD, f32)
    q  = (alpha*d)^2                          (ACT Square, f32, in-place)
    w  = exp(-q + ln(spatial)) -> bf16        (ACT Exp)
    mm = w * x[n+delta]  (bf16)   uu = w * x[n]  (bf16)
    PSUM[wx] += mm[center] + uu[shifted]      (PE identity-matmul, f32 accum)
    PSUM[w]  += w[center]  + w[shifted]       (PE identity-matmul)

then out = (PSUM[wx] + x_center) * recip(PSUM[w] + 1 + eps); the center tap
(w == 1 exactly) is folded into the epilogue. bf16 appears only in the
weights/products (absmax error ~2e-3, max pointwise rel err ~4e-3);
differences, squares, exp inputs and all accumulation stay f32.
"""

import sys

for _p in ("/opt/trn_rl_repo",):
    if _p not in sys.path:
        sys.path.insert(0, _p)

import math
import numpy as np
from numpy.lib.stride_tricks import as_strided

KS = 5
PAD = KS // 2
SIGMA_RANGE = 0.1
EPS = 1e-8
B, C, H, W = 4, 3, 512, 512
BLK = 32
SB = BLK + 2 * PAD  # 36
NCORES = 8
NBH = H // BLK  # 16
NBW = W // BLK  # 16
UNITS = B * NBH * NBW  # 1024
UPC = UNITS // NCORES  # 128 = partitions per core
GRID = SB * SB  # 1296 per channel

ALPHA = 1.0 / (math.sqrt(2.0) * SIGMA_RANGE)

# 12 "positive" pair offsets (a, b): a in [-2,2], b in [-2,2]
PAIRS = [
    (a, b)
    for a in range(0, PAD + 1)
    for b in range(-PAD, PAD + 1)
    if (a > 0) or (a == 0 and b > 0)
]

TRACE = False
LAST_STATS = {}

_cache = {}

# how many of the 12 pair-subtracts (per channel) go to gpsimd
D_ON_GPSIMD = 8


def _build(sk_flat, repeat=1):
    import concourse.bacc as bacc
    import concourse.tile as tile
    from concourse import mybir
    from contextlib import ExitStack

    f32 = mybir.dt.float32
    bf16 = mybir.dt.bfloat16
    nc = bacc.Bacc(None)
    xs_h = nc.dram_tensor("xs", [UPC, C * GRID], f32, kind="ExternalInput")
    xbe_h = nc.dram_tensor("xbe", [UPC, C * GRID], bf16, kind="ExternalInput")
    xbo_h = nc.dram_tensor("xbo", [UPC, C * GRID], bf16, kind="ExternalInput")
    out_h = nc.dram_tensor("out", [UPC, C * BLK * BLK], f32, kind="ExternalOutput")
    ident_h = nc.inline_tensor(np.eye(UPC, dtype=np.float32), "ident")

    with tile.TileContext(nc) as tc, ExitStack() as ctx:
        xin = ctx.enter_context(tc.tile_pool(name="xin", bufs=1))
        consts = ctx.enter_context(tc.tile_pool(name="consts", bufs=1))
        work = ctx.enter_context(tc.tile_pool(name="work", bufs=4))
        ep = ctx.enter_context(tc.tile_pool(name="ep", bufs=2))
        psum = ctx.enter_context(tc.tile_pool(name="psum", bufs=2, space="PSUM"))

        identf = consts.tile([UPC, UPC], f32, tag="identf", name="identf")
        nc.sync.dma_start(out=identf[:], in_=ident_h[:])
        identb = consts.tile([UPC, UPC], bf16, tag="identb", name="identb")
        nc.vector.tensor_copy(identb[:], identf[:])

        lns_map = {}
        bias_tiles = {}
        for (a, b) in PAIRS:
            v = round(float(np.log(sk_flat[(a + PAD) * KS + (b + PAD)])), 9)
            lns_map[(a, b)] = v
            if v not in bias_tiles:
                bt = consts.tile(
                    [UPC, 1], f32, tag=f"lns{v}", name=f"lns{len(bias_tiles)}"
                )
                nc.vector.memset(bt[:], v)
                bias_tiles[v] = bt

        one_eps = consts.tile([UPC, 1], f32, tag="one_eps", name="one_eps")
        nc.vector.memset(one_eps[:], 1.0 + EPS)

        x_t = xin.tile([UPC, C, SB, SB], f32, name="x_t")
        nc.sync.dma_start(
            out=x_t[:].rearrange("p a b c -> p (a b c)"), in_=xs_h[:]
        )
        xbe = xin.tile([UPC, C, SB, SB], bf16, name="xbe")
        nc.sync.dma_start(
            out=xbe[:].rearrange("p a b c -> p (a b c)"), in_=xbe_h[:]
        )
        xbo = xin.tile([UPC, C, SB, SB], bf16, name="xbo")
        nc.sync.dma_start(
            out=xbo[:].rearrange("p a b c -> p (a b c)"), in_=xbo_h[:]
        )

        for _rep in range(repeat):
            o_full = ep.tile([UPC, C, BLK, BLK], f32, tag="o_full", name=f"of{_rep}")
            for ch in range(C):
                xg = x_t[:, ch]  # [UPC, 36, 36] f32 grid view
                pwx = psum.tile(
                    [UPC, BLK * BLK], f32, tag="pwx", name=f"pwx{_rep}_{ch}"
                )
                pw = psum.tile(
                    [UPC, BLK * BLK], f32, tag="pw", name=f"pw{_rep}_{ch}"
                )
                for pi, (a, b) in enumerate(PAIRS):
                    # tight region: union of the center window [2,34)^2 and
                    # the shifted window [2-a,34-a)x[2-b,34-b) -- everything
                    # outside it is never read downstream
                    r0, r1 = PAD - a, PAD + BLK
                    c0 = PAD - max(0, b)
                    c1 = PAD + BLK - min(0, b)

                    dq = work.tile(
                        [UPC, SB, SB], f32, tag="dq", name=f"d{_rep}_{ch}_{pi}"
                    )
                    deng = nc.gpsimd if pi < D_ON_GPSIMD else nc.vector
                    deng.tensor_sub(
                        dq[:, r0:r1, c0:c1],
                        xg[:, r0 + a : r1 + a, c0 + b : c1 + b],
                        xg[:, r0:r1, c0:c1],
                    )
                    nc.scalar.activation(
                        dq[:, r0:r1, c0:c1],
                        dq[:, r0:r1, c0:c1],
                        mybir.ActivationFunctionType.Square,
                        scale=ALPHA,
                    )
                    w = work.tile(
                        [UPC, SB, SB], bf16, tag="w", name=f"w{_rep}_{ch}_{pi}"
                    )
                    nc.scalar.activation(
                        w[:, r0:r1, c0:c1],
                        dq[:, r0:r1, c0:c1],
                        mybir.ActivationFunctionType.Exp,
                        bias=bias_tiles[lns_map[(a, b)]][:],
                        scale=-1.0,
                    )

                    # bf16 x view with even start parity for column offset
                    def xb_at(rr0, rr1, cb0, width):
                        if cb0 % 2 == 0:
                            return xbe[:, ch, rr0:rr1, cb0 : cb0 + width]
                        return xbo[:, ch, rr0:rr1, cb0 - 1 : cb0 - 1 + width]

                    # mm = w * x[n+delta] on the center window
                    mm = work.tile(
                        [UPC, BLK, BLK], bf16, tag="mm", name=f"mm{_rep}_{ch}_{pi}"
                    )
                    nc.vector.tensor_mul(
                        mm[:],
                        w[:, PAD : PAD + BLK, PAD : PAD + BLK],
                        xb_at(PAD + a, PAD + a + BLK, PAD + b, BLK),
                    )
                    # uu = w * x[n] on the shifted window; for odd column
                    # starts, widen by 2 so the bf16 reads stay 4B-aligned
                    # (keeps the DVE 2x perf mode), and let the PE read the
                    # interior 32 columns.
                    sr = PAD - a
                    sc = PAD - b
                    uo = sc % 2  # 1 when widened
                    uw = BLK + 2 * uo
                    uu = work.tile(
                        [UPC, BLK, BLK + 2], bf16, tag="uu",
                        name=f"uu{_rep}_{ch}_{pi}"
                    )
                    nc.vector.tensor_mul(
                        uu[:, :, :uw],
                        w[:, sr : sr + BLK, sc - uo : sc - uo + uw],
                        xb_at(sr, sr + BLK, sc - uo, uw),
                    )

                    # PE accumulation: two 512-column halves per target
                    first = pi == 0
                    last = pi == len(PAIRS) - 1
                    hb = BLK // 2
                    for h in range(2):
                        rows = slice(h * hb, (h + 1) * hb)
                        cols = slice(h * 512, (h + 1) * 512)
                        nc.tensor.matmul(
                            pwx[:, cols], identb[:], mm[:, rows],
                            start=first, stop=False,
                        )
                        nc.tensor.matmul(
                            pwx[:, cols], identb[:], uu[:, rows, uo : uo + BLK],
                            start=False, stop=last,
                        )
                        nc.tensor.matmul(
                            pw[:, cols], identb[:],
                            w[:, PAD + h * hb : PAD + (h + 1) * hb,
                              PAD : PAD + BLK],
                            start=first, stop=False,
                        )
                        nc.tensor.matmul(
                            pw[:, cols], identb[:],
                            w[:, sr + h * hb : sr + (h + 1) * hb,
                              sc : sc + BLK],
                            start=False, stop=last,
                        )

                # epilogue for this channel
                aw = ep.tile([UPC, BLK * BLK], f32, tag="aw", name=f"aw{_rep}_{ch}")
                nc.scalar.activation(
                    aw[:], pw[:], mybir.ActivationFunctionType.Identity,
                    bias=one_eps[:],
                )
                rr = ep.tile([UPC, BLK * BLK], f32, tag="rr", name=f"rr{_rep}_{ch}")
                nc.vector.reciprocal_approx_fast(rr[:], aw[:])
                wxs = ep.tile(
                    [UPC, BLK * BLK], f32, tag="wxs", name=f"wxs{_rep}_{ch}"
                )
                nc.vector.tensor_add(
                    wxs[:].rearrange("p (a b) -> p a b", a=BLK),
                    pwx[:].rearrange("p (a b) -> p a b", a=BLK),
                    xg[:, PAD : PAD + BLK, PAD : PAD + BLK],
                )
                nc.vector.tensor_mul(
                    o_full[:, ch].rearrange("p a b -> p (a b)"), wxs[:], rr[:]
                )
            nc.sync.dma_start(
                out=out_h[:], in_=o_full[:].rearrange("p a b c -> p (a b c)")
            )
    nc.finalize()
    return nc


def _shard(x):
    xp = np.pad(x, ((0, 0), (0, 0), (PAD, PAD), (PAD, PAD)), mode="reflect")
    xp = np.ascontiguousarray(xp)
    sb, sc, sh, sw = xp.strides
    v = as_strided(
        xp,
        shape=(B, NBH, NBW, C, SB, SB),
        strides=(sb, BLK * sh, BLK * sw, sc, sh, sw),
    )
    return np.ascontiguousarray(v).reshape(NCORES, UPC, C * GRID)


def _unshard(outs):
    o = outs.reshape(B, NBH, NBW, C, BLK, BLK)
    return np.ascontiguousarray(o.transpose(0, 3, 1, 4, 2, 5).reshape(B, C, H, W))


def _inputs_for(x):
    import ml_dtypes

    shards = _shard(x)  # (8, 128, C*GRID) f32
    xbe = shards.astype(ml_dtypes.bfloat16)
    xbo = np.empty_like(xbe)
    xbo[:, :, :-1] = xbe[:, :, 1:]
    xbo[:, :, -1] = 0
    return shards, xbe, xbo


def _pjrt_parts(nc):
    """Mirror bass2jax.run_bass_via_pjrt's signature extraction."""
    from concourse import bass2jax, mybir
    import jax

    bass2jax.install_neuronx_cc_hook()
    partition_name = nc.partition_id_tensor.name if nc.partition_id_tensor else None
    in_names, out_names, out_avals, zero_outs = [], [], [], []
    for alloc in nc.m.functions[0].allocations:
        if not isinstance(alloc, mybir.MemoryLocationSet):
            continue
        name = alloc.memorylocations[0].name
        if alloc.kind == "ExternalInput":
            if name != partition_name:
                in_names.append(name)
        elif alloc.kind == "ExternalOutput":
            shape = tuple(alloc.tensor_shape)
            dtype = mybir.dt.np(alloc.dtype)
            out_names.append(name)
            out_avals.append(jax.core.ShapedArray(shape, dtype))
            zero_outs.append(np.zeros(shape, dtype))
    return partition_name, in_names, out_names, out_avals, zero_outs


def _make_runner(nc):
    """jit-compiled SPMD callable for this nc."""
    import jax
    from jax.experimental.shard_map import shard_map
    from jax.sharding import Mesh, NamedSharding, PartitionSpec
    from concourse import bass2jax

    pname, in_names, out_names, out_avals, zero_outs = _pjrt_parts(nc)
    n_params = len(in_names)
    all_in_names = list(in_names) + list(out_names)
    if pname is not None:
        all_in_names.append(pname)

    def _body(*args):
        operands = list(args)
        if pname is not None:
            operands.append(bass2jax.partition_id_tensor())
        return tuple(
            bass2jax._bass_exec_p.bind(
                *operands,
                out_avals=tuple(out_avals),
                in_names=tuple(all_in_names),
                out_names=tuple(out_names),
                lowering_input_output_aliases=(),
                sim_require_finite=True,
                sim_require_nnan=True,
                nc=nc,
            )
        )

    devices = jax.devices()[:NCORES]
    mesh = Mesh(np.asarray(devices), ("core",))
    spec = PartitionSpec("core")
    n_outs = len(out_names)
    fn = jax.jit(
        shard_map(
            _body,
            mesh=mesh,
            in_specs=(spec,) * (n_params + n_outs),
            out_specs=(spec,) * n_outs,
            check_rep=False,
        ),
        keep_unused=True,
    )
    sh = NamedSharding(mesh, spec)
    return fn, sh, in_names, out_avals, zero_outs


def sim_estimate(nc):
    from concourse.timeline_sim import TimelineSim

    return TimelineSim(nc, no_exec=True).simulate()


def _dev_inputs(x, sh, in_names, zero_outs):
    import jax

    shards, xbe, xbo = _inputs_for(x)
    arrs = {
        "xs": shards.reshape(NCORES * UPC, C * GRID),
        "xbe": xbe.reshape(NCORES * UPC, C * GRID),
        "xbo": xbo.reshape(NCORES * UPC, C * GRID),
    }
    dev = [jax.device_put(arrs[nm], sh) for nm in in_names]
    dev += [
        jax.device_put(np.zeros((NCORES * z.shape[0], *z.shape[1:]), z.dtype), sh)
        for z in zero_outs
    ]
    return dev


def bench(x, spatial_kernel, rep_lo=11, rep_hi=41, reps=16):
    """Marginal per-iteration device time: interleaved timing of two
    repeat-NEFFs of similar size (fixed dispatch cost cancels)."""
    import time
    import jax

    x = np.ascontiguousarray(np.asarray(x, dtype=np.float32))
    sk = np.asarray(spatial_kernel, dtype=np.float64).reshape(-1)
    key = sk.tobytes()
    if key not in _cache:
        _cache[key] = _build(sk)
    nc1 = _cache[key]

    runners = {}
    for n in (rep_lo, rep_hi):
        key_r = (key, n)
        if key_r not in _cache:
            _cache[key_r] = _build(sk, repeat=n)
        fn, sh, in_names, out_avals, zero_outs = _make_runner(_cache[key_r])
        dev_in = _dev_inputs(x, sh, in_names, zero_outs)
        jax.block_until_ready(fn(*dev_in))
        runners[n] = (fn, dev_in)

    # correctness output from the single-shot program
    fn1, sh1, in_names1, out_avals1, zero_outs1 = _make_runner(nc1)
    dev_in1 = _dev_inputs(x, sh1, in_names1, zero_outs1)
    outs = fn1(*dev_in1)
    jax.block_until_ready(outs)
    outs_np = np.asarray(outs[0]).reshape(NCORES, UPC, C, BLK, BLK)

    # per-dispatch overhead is bimodal (~40ms vs ~78ms); medians are stable
    samples = {n: [] for n in runners}
    for _ in range(reps):
        for n, (fn, dev_in) in runners.items():
            t0 = time.perf_counter()
            jax.block_until_ready(fn(*dev_in))
            samples[n].append(time.perf_counter() - t0)
    med = {n: float(np.median(np.asarray(t))) for n, t in samples.items()}
    marg_ns = (med[rep_hi] - med[rep_lo]) / (rep_hi - rep_lo) * 1e9
    stats = {
        "chain_ns": marg_ns,
        f"t_r{rep_lo}": med[rep_lo],
        f"t_r{rep_hi}": med[rep_hi],
        "sim_r1_ns": sim_estimate(nc1),
    }
    full = _unshard(outs_np.astype(np.float32))
    return stats, full


def kernel(x, spatial_kernel):
    import jax
    from concourse.bass_utils import run_bass_kernel_spmd

    x = np.ascontiguousarray(np.asarray(x, dtype=np.float32))
    sk = np.asarray(spatial_kernel, dtype=np.float64).reshape(-1)

    key = sk.tobytes()
    if key not in _cache:
        _cache[key] = _build(sk)
    nc = _cache[key]

    rkey = (key, "runner")
    if rkey in _cache:
        # repeat calls: reuse the jitted SPMD executable (same bass2jax
        # execution path as run_bass_kernel_spmd under axon, minus the
        # per-call re-trace)
        fn, sh, in_names, out_avals, zero_outs = _cache[rkey]
        dev_in = _dev_inputs(x, sh, in_names, zero_outs)
        outs = fn(*dev_in)
        jax.block_until_ready(outs)
        out_np = np.asarray(outs[0]).reshape(NCORES, UPC, C, BLK, BLK)
        return _unshard(out_np.astype(np.float32))

    shards, xbe, xbo = _inputs_for(x)
    in_maps = [
        {"xs": shards[c], "xbe": xbe[c], "xbo": xbo[c]} for c in range(NCORES)
    ]
    res = run_bass_kernel_spmd(nc, in_maps, list(range(NCORES)), trace=TRACE)
    LAST_STATS.clear()
    LAST_STATS.update(
        exec_time_ns=res.exec_time_ns,
        mean_exec_time_ns=res.mean_exec_time_ns,
    )
    _cache[rkey] = _make_runner(nc)
    outs = np.stack([r["out"] for r in res.results]).astype(np.float32)
    return _unshard(outs.reshape(NCORES, UPC, C, BLK, BLK))
